# revision 65
# baseline (speedup 1.0000x reference)
"""MLA (DeepSeek-style) attention block on 8 Trainium2 NeuronCores.

Sharding:
  phase 1 (token-parallel, 8 x 512 tokens): LoRA-A down-projections + rmsnorm
    + k_pe rope; small AllGather of the kv latents (576 dims); q up-projection
    for ALL 16 heads on the token side + rope, shipped to head shards via
    three shard-aligned AllToAlls (pe2, then h0-nope, then h1-nope).
  phase 2 (head-parallel, 2 heads x 2 batches per core): k/v up-projection
    from gathered kv latents; causal flash attention, k-major.
  output: two AllToAlls (even heads overlap the odd-head attention; odd heads
    overlap the even half of the token-parallel output projection).

PE-offload choices (the tensor engine is the bottleneck; its duty is further
capped ~81% by the activity governor during dense streams):
  - causal mask as a 0/1 keep-mask MULTIPLY on GpSimd (not a PE matmul);
    diagonal chunks also shrink scores/exp to their live columns
  - softmax denominator via bf16 tree-sums on VectorE + ONE matmul per qtile
  - all [1,N]->[128,N] broadcast matmuls in bf16 (an fp32 matmul is a
    multi-pass ~1.3us instruction)
  - adjacent k-chunks' half-width (64-row) kpe score matmuls are emitted
    back-to-back into DISJOINT PE row-groups so they execute concurrently
    (q_pe is duplicated per head-half into rows 0-63/64-127 for this)
  - PV runs four chunks behind scores; the per-qtile normalize matmuls are
    deferred into the next qtile's chunk stream: the in-order PE queue
    never waits on the exp (ScalarE) -> keep-mask (GpSimd) chain

DMA: the big weight prefetches ride the two hardware-DGE rings (sync +
scalar) ordered so ring backpressure never blocks a compute chain: hid/wkva
chunk pairs first, wqa split across both rings, wqb after the AllGather
trigger.  The gpsimd SWDGE ring only carries small/late traffic.
bf16 matmuls, fp32 PSUM accumulation + softmax statistics, fp32 output.
"""
import sys
from contextlib import ExitStack

sys.path.insert(0, "/opt/trn_rl_repo")

import numpy as np
import ml_dtypes

import concourse.bacc as bacc
import concourse.mybir as mybir
import concourse.tile as tile
from concourse.bass_utils import run_bass_kernel_spmd

# ---- problem sizes (hardcoded per spec) ----
HID = 2048; H = 16; QLR = 1536; KVLR = 512
DN = 128; DR = 64; DV = 128; DQ = DN + DR
B = 2; S = 2048
THETA = 10000.0; EPS = 1e-6

NCORES = 8
T = B * S              # 4096 flattened tokens
TPC = T // NCORES      # 512 tokens per core
HPC = H // NCORES      # 2 heads per core
P = 128
NHID = HID // P        # 16
NQLR = QLR // P        # 12
CKW = KVLR + DR        # 576
QT_PER_B = S // 512    # 4 q-tiles of 512 per (b,h) unit
KB_PER_B = S // P      # 16 k-chunks of 128 per batch
WKK = HPC * DN         # 256
WQB = H * DQ           # 3072

BF16 = mybir.dt.bfloat16
F32 = mybir.dt.float32
AF = mybir.ActivationFunctionType

_NC_CACHE = None


def _rope_dual(nc, pool, out_bf16, ps, cos_sb, sin_sb, tag):
    """RoPE on a [128, W] psum holding two 64-row head groups; writes bf16."""
    W = 512
    HDR = DR // 2
    rot = pool.tile([P, W], F32, tag=f"{tag}rot", name=f"{tag}rot")
    for g in range(2):
        o = g * DR
        nc.scalar.mul(rot[o:o + HDR, :], ps[o + HDR:o + DR, :], -1.0)
        nc.scalar.copy(rot[o + HDR:o + DR, :], ps[o:o + HDR, :])
    t1 = pool.tile([P, W], F32, tag=f"{tag}t1", name=f"{tag}t1")
    nc.vector.tensor_mul(t1[:], ps[:], cos_sb[:])
    nc.vector.tensor_mul(rot[:], rot[:], sin_sb[:])
    nc.vector.tensor_add(out_bf16[:], t1[:], rot[:])


def _phase1ab(nc, tc, hidT, wqaT, wkvaT, latkv_in, latkv_all,
              cq_bf, rn_q, ones_col, ones_row, ones1, eps_t, cos_sb, sin_sb, RG,
              after_wqa_dma=None):
    """ckv path (+ kv AllGather) then cq path; SBUF freed on exit.

    ckv is kc-outer (5 psum accumulators) so the tensor engine starts as
    soon as the first hid/wkva chunk pair lands.  All row-sum (ssq) matmuls
    are deferred one block so the in-order PE queue never waits on the
    scalar-copy -> square chain."""
    with tc.tile_pool(name="p1a", bufs=1) as p1a, \
         tc.tile_pool(name="p1t", bufs=2) as p1t, \
         tc.tile_pool(name="p1n", bufs=1) as p1n:
        wqa_sb = p1a.tile([P, NHID * QLR], BF16)
        hid_ch = [p1a.tile([P, TPC], BF16, tag=f"hid{kc}", name=f"hid{kc}")
                  for kc in range(NHID)]
        wkva_ch = [p1a.tile([P, CKW], BF16, tag=f"wkva{kc}", name=f"wkva{kc}")
                   for kc in range(NHID)]
        # critical first: per-chunk hid+wkva pairs, issue split across the
        # sync and scalar queues (each dma_start costs ~800ns of engine time)
        for kc in range(NHID):
            eng = nc.sync if kc % 2 == 0 else nc.scalar
            eng.dma_start(hid_ch[kc][:], hidT.ap()[kc * P:(kc + 1) * P, :])
            eng.dma_start(wkva_ch[kc][:], wkvaT.ap()[kc * P:(kc + 1) * P, :])
        # wqa split across BOTH hardware-DGE rings (sync+scalar) behind the
        # hid/wkva pairs so it lands by ~20us for cq_block(0)
        for g in range(4):
            eng = nc.sync if g % 2 == 0 else nc.scalar
            eng.dma_start(
                wqa_sb[:, g * 4 * QLR:(g + 1) * 4 * QLR]
                .rearrange("p (k c) -> p k c", k=4),
                wqaT.ap()[g * 4 * P:(g + 1) * 4 * P, :]
                .rearrange("(k p) c -> p k c", p=P))

        # --- ckv joint, kc-outer: 4 normed blocks + k_pe accumulate together ---
        with tc.tile_pool(name="p1ckv", bufs=1) as p1ckv:
            ckv_bf = p1ckv.tile([P, 4 * TPC], BF16)
            with tc.tile_pool(name="psA", bufs=1, space="PSUM") as psA:
                ps_m = [psA.tile([P, TPC], F32, tag=f"ckv{m}", name=f"ckv{m}")
                        for m in range(4)]
                ps_pe = psA.tile([DR, TPC], F32, tag="ckvpe", name="ckvpe")
                for kc in range(NHID):
                    for m in range(4):
                        nc.tensor.matmul(ps_m[m][:],
                                         wkva_ch[kc][:, m * P:(m + 1) * P],
                                         hid_ch[kc][:],
                                         start=(kc == 0), stop=(kc == NHID - 1),
                                         skip_group_check=True)
                    nc.tensor.matmul(ps_pe[:], wkva_ch[kc][:, KVLR:CKW],
                                     hid_ch[kc][:],
                                     start=(kc == 0), stop=(kc == NHID - 1),
                                     skip_group_check=True)

                # k_pe rope (shared across heads) -> latkv rows KVLR:CKW
                HDR = DR // 2
                rot = p1t.tile([DR, TPC], F32, tag="rot")
                nc.scalar.mul(rot[0:HDR, :], ps_pe[HDR:DR, :], -1.0)
                nc.scalar.copy(rot[HDR:DR, :], ps_pe[0:HDR, :])
                t1 = p1t.tile([DR, TPC], F32, tag="t1")
                nc.vector.tensor_mul(t1[:], ps_pe[:], cos_sb[0:DR, :])
                nc.vector.tensor_mul(rot[:], rot[:], sin_sb[0:DR, :])
                pe_out = p1t.tile([DR, TPC], BF16, tag="peo")
                nc.vector.tensor_add(pe_out[:], t1[:], rot[:])
                nc.sync.dma_start(latkv_in[KVLR:CKW, :], pe_out[:])

                for m in range(4):
                    nc.scalar.copy(ckv_bf[:, m * TPC:(m + 1) * TPC], ps_m[m][:])

            # --- cq blocks interleaved with the ckv norm tail; ssq matmuls
            # --- deferred one block behind their squares ---
            with tc.tile_pool(name="ps1", bufs=4, space="PSUM") as ps1, \
                 tc.tile_pool(name="ps1s", bufs=1, space="PSUM") as ps1s, \
                 tc.tile_pool(name="ps1b", bufs=2, space="PSUM") as ps1b:
                ssq_q = ps1s.tile([1, TPC], F32, tag="sq")
                ssq_kv = ps1s.tile([1, TPC], F32, tag="skv")
                pend_ssq = []

                # consume wqa kc-groups in their DMA arrival order (sync-ring
                # groups 0/2 land before scalar-ring groups 1/3): the psum
                # accumulation is commutative, and block 0 stops waiting on
                # the last-landing group
                CQ_ORDER = [0, 1, 2, 3, 8, 9, 10, 11, 4, 5, 6, 7, 12, 13, 14, 15]

                def cq_block(m):
                    ps = ps1.tile([P, TPC], F32, tag="proj")
                    for i, kc in enumerate(CQ_ORDER):
                        nc.tensor.matmul(ps[:], wqa_sb[:, kc * QLR + m * P:
                                                       kc * QLR + (m + 1) * P],
                                         hid_ch[kc][:],
                                         start=(i == 0), stop=(i == NHID - 1))
                    for f in pend_ssq:
                        f()
                    pend_ssq.clear()
                    nc.scalar.copy(cq_bf[:, m * TPC:(m + 1) * TPC], ps[:])
                    sq = p1t.tile([P, TPC], BF16, tag="sq")
                    nc.vector.tensor_mul(sq[:], cq_bf[:, m * TPC:(m + 1) * TPC],
                                         cq_bf[:, m * TPC:(m + 1) * TPC])
                    pend_ssq.append(lambda sq=sq, m=m: nc.tensor.matmul(
                        ssq_q[:], ones_col[:], sq[:],
                        start=(m == 0), stop=(m == NQLR - 1),
                        skip_group_check=True))

                cq_block(0)
                # ckv squares on vector; their row-sum matmuls are DEFERRED
                # into cq_block(2)'s stream so the in-order PE queue never
                # waits on the scalar-copy -> square chain
                sqs = []
                for m in range(4):
                    sq = p1n.tile([P, TPC], BF16, tag=f"csq{m}")
                    nc.vector.tensor_mul(sq[:], ckv_bf[:, m * TPC:(m + 1) * TPC],
                                         ckv_bf[:, m * TPC:(m + 1) * TPC])
                    sqs.append(sq)
                cq_block(1)
                # ssq_kv row-sums follow cq1's matmuls in the PE queue: the
                # squares (vector) complete while cq1 streams -> no stall,
                # and the latents ship ~8us earlier than deferring past cq3
                for m in range(4):
                    nc.tensor.matmul(ssq_kv[:], ones_col[:], sqs[m][:],
                                     start=(m == 0), stop=(m == 3),
                                     skip_group_check=True)
                # norm chain sits after cq1's copy in the scalar queue, by
                # which time its ssq_kv inputs are done -> no head blocking
                kv_norm = p1n.tile([1, TPC], F32, tag="nrm")
                nc.scalar.activation(kv_norm[:], ssq_kv[:], AF.Sqrt, bias=eps_t[:],
                                     scale=1.0 / KVLR)
                rn_kv = p1n.tile([1, TPC], F32, tag="rn")
                nc.vector.reciprocal(rn_kv[:], kv_norm[:])
                rn_kv_bf = p1n.tile([1, TPC], BF16, tag="rnb")
                nc.gpsimd.tensor_mul(rn_kv_bf[:], rn_kv[:], ones1[:])
                bkv = ps1b.tile([P, TPC], F32, tag="bc")
                nc.tensor.matmul(bkv[:], ones_row[:], rn_kv_bf[:], start=True, stop=True)
                for m in range(4):
                    lat_sb = p1t.tile([P, TPC], BF16, tag="lat")
                    nc.vector.tensor_mul(lat_sb[:], ckv_bf[:, m * TPC:(m + 1) * TPC], bkv[:])
                    nc.sync.dma_start(latkv_in[m * P:(m + 1) * P, :], lat_sb[:])
                nc.gpsimd.collective_compute(
                    "AllGather", mybir.AluOpType.bypass, replica_groups=RG,
                    ins=[latkv_in.opt()], outs=[latkv_all.opt()])
                # wqb rides the sync ring BEHIND the latent stores + trigger,
                # so its backpressure cannot delay the AllGather
                if after_wqa_dma is not None:
                    after_wqa_dma()

                for m in range(2, NQLR):
                    cq_block(m)
                for f in pend_ssq:
                    f()
                pend_ssq.clear()
                sq_norm = p1n.tile([1, TPC], F32, tag="nrm")
                nc.scalar.activation(sq_norm[:], ssq_q[:], AF.Sqrt, bias=eps_t[:],
                                     scale=1.0 / QLR)
                # 1/rms is NOT applied to cq here: it is a per-token scalar, so
                # it commutes through the q up-projection and is folded into
                # phase 1c's rope tables / output muls instead.
                nc.vector.reciprocal(rn_q[:], sq_norm[:])


def build_nc():
    nc = bacc.Bacc(None, target_bir_lowering=False, debug=False, num_devices=NCORES)

    # ---- per-core external inputs ----
    hidT = nc.dram_tensor("hidT", [HID, TPC], BF16, kind="ExternalInput")
    wqaT = nc.dram_tensor("wqaT", [HID, QLR], BF16, kind="ExternalInput")
    wkvaT = nc.dram_tensor("wkvaT", [HID, CKW], BF16, kind="ExternalInput")
    wqbT = nc.dram_tensor("wqbT", [QLR, WQB], BF16, kind="ExternalInput")
    wkvbkT = nc.dram_tensor("wkvbkT", [KVLR, HPC * DN], BF16, kind="ExternalInput")
    wkvbvT = nc.dram_tensor("wkvbvT", [KVLR, HPC * DV], BF16, kind="ExternalInput")
    woT = nc.dram_tensor("woT", [H * DV, HID], BF16, kind="ExternalInput")
    cosd = nc.dram_tensor("cosd", [P, TPC], F32, kind="ExternalInput")
    sind = nc.dram_tensor("sind", [P, TPC], F32, kind="ExternalInput")
    # 4 multiplicative (0/1) causal keep-masks
    masks = nc.dram_tensor("masks", [P, 4 * 512], BF16, kind="ExternalInput")
    outT = nc.dram_tensor("outT", [HID, TPC], F32, kind="ExternalOutput")

    RG = [list(range(NCORES))]

    with tile.TileContext(nc) as tc:
        with tc.tile_pool(name="dram", bufs=1, space="DRAM") as dram, \
             tc.tile_pool(name="const", bufs=1) as const:
            latkv_in = dram.tile([CKW, TPC], BF16)
            latkv_all = dram.tile([NCORES * CKW, TPC], BF16, addr_space="Shared")
            qa1_in = dram.tile([NCORES * P, TPC], BF16)   # pe2 per pair
            qa1_out = dram.tile([NCORES * P, TPC], BF16)
            qa2_in = dram.tile([NCORES * P, TPC], BF16)   # h0 nope per pair
            qa2_out = dram.tile([NCORES * P, TPC], BF16)
            qb_in = dram.tile([NCORES * P, TPC], BF16)    # h1 nope per pair
            qb_out = dram.tile([NCORES * P, TPC], BF16)
            oa_in = dram.tile([NCORES * DV, TPC], BF16)   # even heads out
            oa_out = dram.tile([NCORES * DV, TPC], BF16)
            ob_in = dram.tile([NCORES * DV, TPC], BF16)   # odd heads out
            ob_out = dram.tile([NCORES * DV, TPC], BF16)

            ones_col = const.tile([P, 1], BF16)
            nc.vector.memset(ones_col[:], 1.0)
            # bf16 ones_row: keeps the [1,N] -> [128,N] broadcast matmuls on the
            # fast bf16 path (an fp32 matmul is a 2x2-pass ~1.3us instruction)
            ones_row = const.tile([1, P], BF16)
            nc.vector.memset(ones_row[:], 1.0)
            ones1 = const.tile([1, 512], BF16)
            nc.vector.memset(ones1[:], 1.0)
            ones_full = const.tile([P, 512], BF16)
            nc.vector.memset(ones_full[:], 1.0)
            eps_t = const.tile([1, 1], F32)
            nc.vector.memset(eps_t[:], EPS)

            # ============ Phase 1: token-parallel compute ============
            p1q_stack = ExitStack()
            p1q = p1q_stack.enter_context(tc.tile_pool(name="p1q", bufs=1))
            if True:
                # rope tables live only through phase 1; scoping them here
                # (not in const) frees 4KB/partition for a deeper kv-up
                # double-buffer during attention
                cos_sb = p1q.tile([P, TPC], F32)
                sin_sb = p1q.tile([P, TPC], F32)
                nc.sync.dma_start(cos_sb[:], cosd.ap()[:])
                nc.sync.dma_start(sin_sb[:], sind.ap()[:])
                wqb_sb = p1q.tile([P, NQLR * WQB], BF16)
                cq_bf = p1q.tile([P, NQLR * TPC], BF16)
                rn_q = p1q.tile([1, TPC], F32)
                cosq = p1q.tile([P, TPC], F32)
                sinq = p1q.tile([P, TPC], F32)
                bq_sb = p1q.tile([P, TPC], F32)

                def _wqb_dma():
                    # wqb on the sync hardware-DGE ring (emitted after the
                    # AllGather trigger; sync has no further work until 1c)
                    for g in range(6):
                        eng = nc.sync
                        eng.dma_start(
                            wqb_sb[:, g * 2 * WQB:(g + 1) * 2 * WQB]
                            .rearrange("p (k c) -> p k c", k=2),
                            wqbT.ap()[g * 2 * P:(g + 1) * 2 * P, :]
                            .rearrange("(k p) c -> p k c", p=P))
                # ---- phase 1a/1b scope (freed before q up-projection) ----
                _phase1ab(nc, tc, hidT, wqaT, wkvaT,
                          latkv_in, latkv_all, cq_bf, rn_q, ones_col, ones_row,
                          ones1, eps_t, cos_sb, sin_sb, RG, after_wqa_dma=_wqb_dma)
                # ============ Phase 1c: q up-projection for ALL heads ============
                # column order of wqbT: 8 pe2 pair-blocks | 8 h0-nope | 8 h1-nope
                with tc.tile_pool(name="p1qt", bufs=3) as p1qt, \
                     tc.tile_pool(name="ps1c", bufs=3, space="PSUM") as ps1c, \
                     tc.tile_pool(name="ps1cb", bufs=1, space="PSUM") as ps1cb:
                    # broadcast 1/rms(q) and fold it into the rope tables /
                    # output muls (it commutes through the up-projection)
                    rnq_bf = p1qt.tile([1, TPC], BF16, tag="rqb")
                    nc.gpsimd.tensor_mul(rnq_bf[:], rn_q[:], ones1[:])
                    bq_bc = ps1cb.tile([P, TPC], F32)
                    nc.tensor.matmul(bq_bc[:], ones_row[:], rnq_bf[:],
                                     start=True, stop=True)
                    nc.scalar.copy(bq_sb[:], bq_bc[:])
                    nc.vector.tensor_mul(cosq[:], cos_sb[:], bq_sb[:])
                    nc.vector.tensor_mul(sinq[:], sin_sb[:], bq_sb[:])
                    for grp, (a2a_in, a2a_out) in enumerate(
                            [(qa1_in, qa1_out), (qa2_in, qa2_out), (qb_in, qb_out)]):
                        for mb in range(8):
                            col = grp * 8 + mb
                            ps = ps1c.tile([P, TPC], F32, tag="proj")
                            for kc in range(NQLR):
                                nc.tensor.matmul(
                                    ps[:], wqb_sb[:, kc * WQB + col * P:
                                                  kc * WQB + (col + 1) * P],
                                    cq_bf[:, kc * TPC:(kc + 1) * TPC],
                                    start=(kc == 0), stop=(kc == NQLR - 1))
                            qo = p1qt.tile([P, TPC], BF16, tag="qo")
                            if grp == 0:  # pe2 block -> rope (rms-scaled tables)
                                _rope_dual(nc, p1qt, qo, ps, cosq, sinq, "q")
                            else:
                                nc.vector.tensor_mul(qo[:], ps[:], bq_sb[:])
                            # last group ships via the scalar ring so the
                            # kv-up latkv loads are not stuck behind it
                            eng = nc.scalar if grp == 2 else nc.sync
                            eng.dma_start(a2a_in[mb * P:(mb + 1) * P, :], qo[:])
                        nc.gpsimd.collective_compute(
                            "AllToAll", mybir.AluOpType.bypass, replica_groups=RG,
                            ins=[a2a_in.opt()], outs=[a2a_out.opt()])
            p1q_stack.close()

            # ===== Phase 2: k/v up-proj + q receive (overlaps the q AllToAlls) =====
            with tc.tile_pool(name="att_a", bufs=1) as att_a, \
                 tc.tile_pool(name="attc", bufs=1) as attc, \
                 tc.tile_pool(name="att_t", bufs=10) as att_t, \
                 tc.tile_pool(name="att_s", bufs=2) as att_s, \
                 tc.tile_pool(name="ps_s", bufs=4, space="PSUM") as ps_s_pool, \
                 tc.tile_pool(name="ps_o", bufs=2, space="PSUM") as ps_o_pool, \
                 tc.tile_pool(name="ps_d", bufs=1, space="PSUM") as ps_d_pool, \
                 tc.tile_pool(name="ps_b", bufs=1, space="PSUM") as ps_b_pool:
                wo_sb = att_a.tile([P, NHID * HID], BF16)  # all w_o, prefetched
                oe_sb = att_a.tile([P, NCORES * TPC], BF16)  # even-head attn out
                mask_sb = attc.tile([P, 4 * 512], BF16)
                nc.sync.dma_start(mask_sb[:], masks.ap()[:])

                att_in_stack = ExitStack()
                att_in = att_in_stack.enter_context(
                    tc.tile_pool(name="att_in", bufs=1))
                att_qp = att_in_stack.enter_context(
                    tc.tile_pool(name="att_qp", bufs=2))
                knope = att_in.tile([P, 2 * T], BF16)
                kpe2 = att_in.tile([P, T], BF16)    # k_pe duplicated rows
                v_sb = att_in.tile([P, (T // P) * WKK], BF16)
                qnope_a = att_in.tile([P, T], BF16)  # h0 nope
                qnope_b = att_in.tile([P, T], BF16)  # h1 nope

                def build_qpe2(hl):
                    # q_pe of head-half hl duplicated into rows 0-63 AND
                    # 64-127 (straight from qa1_out in DRAM), so adjacent
                    # k-chunks' half-width kpe score matmuls can run in
                    # disjoint PE row-groups CONCURRENTLY
                    q2 = att_qp.tile([P, T], BF16, tag="qpe2")
                    src = qa1_out[:].rearrange("(i pp) c -> pp i c", pp=P)
                    for half in range(2):
                        nc.sync.dma_start(
                            q2[half * DR:(half + 1) * DR, :]
                            .rearrange("p (i c) -> p i c", i=NCORES),
                            src[hl * DR:(hl + 1) * DR])
                    return q2

                # qnope_b + w_o on the gpsimd queue (sits right behind the qb
                # AllToAll; gpsimd is otherwise idle until the oa AllToAll)
                for g in range(2):
                    half = NCORES // 2 * TPC
                    hrow = NCORES // 2 * P
                    nc.gpsimd.dma_start(
                        qnope_b[:, g * half:(g + 1) * half]
                        .rearrange("p (i c) -> p i c", i=NCORES // 2),
                        qb_out[g * hrow:(g + 1) * hrow, :]
                        .rearrange("(i p) c -> p i c", p=P))
                # w_o layout: [even-head chunks 0..7 | odd-head chunks 0..7]
                for i in range(NCORES):
                    nc.gpsimd.dma_start(wo_sb[:, i * HID:(i + 1) * HID],
                                        woT.ap()[(2 * i) * P:(2 * i + 1) * P, :])
                for i in range(NCORES):
                    nc.gpsimd.dma_start(
                        wo_sb[:, (NCORES + i) * HID:(NCORES + i + 1) * HID],
                        woT.ap()[(2 * i + 1) * P:(2 * i + 2) * P, :])

                with tc.tile_pool(name="p2w", bufs=1) as p2w, \
                     tc.tile_pool(name="p2a", bufs=4) as p2a:
                    wkk_sb = p2w.tile([P, 4 * WKK], BF16)
                    wkv_sb = p2w.tile([P, 4 * WKK], BF16)
                    nc.gpsimd.dma_start(
                        wkk_sb[:].rearrange("p (k c) -> p k c", k=4),
                        wkvbkT.ap()[:].rearrange("(k p) c -> p k c", p=P))
                    nc.gpsimd.dma_start(
                        wkv_sb[:].rearrange("p (k c) -> p k c", k=4),
                        wkvbvT.ap()[:].rearrange("(k p) c -> p k c", p=P))
                    for j in range(NCORES):
                        basek = j * CKW
                        ckv_j = p2a.tile([P, 4 * TPC], BF16, tag="ckvj")
                        # one batched read per j (4 chunks) instead of 4:
                        # halves the sync-engine issue time and lands earlier
                        nc.sync.dma_start(
                            ckv_j[:].rearrange("p (r c) -> p r c", r=4),
                            latkv_all[basek: basek + KVLR, :]
                            .rearrange("(r p) c -> p r c", p=P))
                        nc.sync.dma_start(kpe2[0:DR, j * TPC:(j + 1) * TPC],
                                          latkv_all[basek + KVLR: basek + CKW, :])
                        nc.sync.dma_start(kpe2[DR:P, j * TPC:(j + 1) * TPC],
                                          latkv_all[basek + KVLR: basek + CKW, :])
                        for m in range(HPC):
                            ps = ps_s_pool.tile([P, TPC], F32, tag="pss")
                            for kc in range(4):
                                nc.tensor.matmul(
                                    ps[:], wkk_sb[:, kc * WKK + m * P: kc * WKK + (m + 1) * P],
                                    ckv_j[:, kc * TPC:(kc + 1) * TPC],
                                    start=(kc == 0), stop=(kc == 3))
                            nc.scalar.copy(knope[:, m * T + j * TPC: m * T + (j + 1) * TPC], ps[:])
                        for tb in range(TPC // P):
                            ps = ps_s_pool.tile([P, WKK], F32, tag="pss")
                            for kc in range(4):
                                nc.tensor.matmul(
                                    ps[:], ckv_j[:, kc * TPC + tb * P: kc * TPC + (tb + 1) * P],
                                    wkv_sb[:, kc * WKK:(kc + 1) * WKK],
                                    start=(kc == 0), stop=(kc == 3))
                            jb = j * (TPC // P) + tb
                            nc.scalar.copy(v_sb[:, jb * WKK:(jb + 1) * WKK], ps[:])

                # ============ attention (4 causal units, hl-major) ============
                if True:
                    for g in range(2):
                        half = NCORES // 2 * TPC
                        hrow = NCORES // 2 * P
                        nc.sync.dma_start(
                            qnope_a[:, g * half:(g + 1) * half]
                            .rearrange("p (i c) -> p i c", i=NCORES // 2),
                            qa2_out[g * hrow:(g + 1) * hrow, :]
                            .rearrange("(i p) c -> p i c", p=P))
                    qpe2h = build_qpe2(0)

                    # deferred-normalize machinery: the qtile finalize is split
                    # in three and emitted inside the NEXT qtile's chunk stream
                    # (fin_den: the single per-qtile denominator matmul over the
                    # vector-accumulated exp sum; fin_a: reciprocal; fin_b:
                    # bf16 broadcast matmul + normalize + store)
                    def make_finishers(hl, bb, qt, ps_o, ps_d, qsum, after_store):
                        blk = bb * QT_PER_B + qt
                        tgt = oa_in if hl == 0 else ob_in
                        st = {}

                        def fin_den():
                            nc.tensor.matmul(ps_d[:], ones_col[:], qsum[:],
                                             start=True, stop=True,
                                             skip_group_check=True)

                        def fin_a():
                            recip32 = att_s.tile([1, 512], F32, tag="rc32")
                            nc.vector.reciprocal_approx_fast(recip32[:], ps_d[:])
                            recip = att_s.tile([1, 512], BF16, tag="rcp")
                            nc.gpsimd.tensor_mul(recip[:], recip32[:], ones1[:])
                            # psum -> sbuf copy of the PV sums (a DVE op may
                            # read at most one PSUM operand, so fin_b's
                            # normalize multiply needs this staged in SBUF)
                            ou = att_s.tile([P, 512], F32, tag="ou")
                            nc.vector.tensor_mul(ou[:], ps_o[:], ones_full[:])
                            st['recip'] = recip
                            st['ou'] = ou

                        def fin_b():
                            bc = ps_b_pool.tile([P, 512], F32, tag="bc")
                            nc.tensor.matmul(bc[:], ones_row[:], st['recip'],
                                             start=True, stop=True,
                                             skip_group_check=True)
                            on = att_s.tile([P, 512], BF16, tag="on")
                            nc.vector.tensor_mul(on[:], st['ou'][:], bc[:])
                            nc.sync.dma_start(tgt[blk * DV:(blk + 1) * DV, :], on[:])
                            if after_store is not None:
                                after_store()

                        return fin_den, fin_a, fin_b

                    # one-time memset of the ex ring: diagonal chunks only
                    # exp-write the live columns, and the keep-mask multiply
                    # must not hit NaN bf16 garbage (NaN*0 = NaN)
                    for _ in range(10):
                        ex0 = att_t.tile([P, 512], BF16, tag="ex")
                        nc.vector.memset(ex0[:], 0.0)

                    fin_den = fin_a = fin_b = None
                    for u in range(4):  # hl-major: (hl, bb)
                        hl, bb = u // 2, u % 2
                        qn = qnope_a if hl == 0 else qnope_b
                        if u == 2:
                            qpe2h = build_qpe2(1)
                        for qt in range(QT_PER_B):
                            qoff = bb * S + qt * 512
                            ps_o = ps_o_pool.tile([P, 512], F32, tag="pso")
                            ps_d = ps_d_pool.tile([1, 512], F32, tag="psd")
                            qsum = att_s.tile([P, 512], BF16, tag="qsum")
                            nkc = 4 * (qt + 1)
                            exs = [None] * nkc
                            # PV runs FOUR chunks behind scores so the
                            # exp (scalar) -> keep-mask (gpsimd) chain never
                            # stalls the in-order PE queue
                            for kc in range(nkc + 4):
                                if kc < nkc and kc % 2 == 0:
                                    # chunk PAIR (kc, kc+1): knope matmuls, then
                                    # the two half-width kpe matmuls back to
                                    # back in disjoint row-groups (0-63 /
                                    # 64-127) so they execute concurrently.
                                    # cols < mi*128 of a diagonal chunk are
                                    # fully masked: skip them in scores/exp
                                    # (matmul out at psum offset 0, bank-
                                    # aligned; exp write shifted to true cols)
                                    pr = []
                                    for sub in range(2):
                                        kcs = kc + sub
                                        koff = bb * S + kcs * P
                                        c0 = (kcs - 4 * qt) * P if kcs >= 4 * qt else 0
                                        w = 512 - c0
                                        ps_sc = ps_s_pool.tile([P, 512], F32, tag="pss")
                                        nc.tensor.matmul(
                                            ps_sc[:, 0:w],
                                            knope[:, hl * T + koff: hl * T + koff + P],
                                            qn[:, qoff + c0: qoff + 512],
                                            start=True, stop=False)
                                        pr.append((kcs, koff, c0, w, ps_sc))
                                    for sub in range(2):
                                        kcs, koff, c0, w, ps_sc = pr[sub]
                                        nc.tensor.matmul(
                                            ps_sc[:, 0:w],
                                            kpe2[sub * DR:(sub + 1) * DR, koff: koff + P],
                                            qpe2h[sub * DR:(sub + 1) * DR,
                                                  qoff + c0: qoff + 512],
                                            start=False, stop=True)
                                    for sub in range(2):
                                        kcs, koff, c0, w, ps_sc = pr[sub]
                                        ex = att_t.tile([P, 512], BF16, tag="ex")
                                        nc.scalar.activation(ex[:, c0:512],
                                                             ps_sc[:, 0:w], AF.Exp)
                                        if kcs >= 4 * qt:
                                            mi = kcs - 4 * qt
                                            if c0 > 0:
                                                nc.gpsimd.memset(ex[:, 0:c0], 0.0)
                                            nc.gpsimd.tensor_mul(
                                                ex[:, c0:512], ex[:, c0:512],
                                                mask_sb[:, mi * 512 + c0:(mi + 1) * 512])
                                        exs[kcs] = ex
                                if kc == 2 and fin_den is not None:
                                    fin_den()
                                    fin_den = None
                                if kc == 3 and fin_a is not None:
                                    fin_a()
                                    fin_a = None
                                if kc == 5 and fin_b is not None:
                                    fin_b()
                                    fin_b = None
                                if kc > 3:
                                    k0 = kc - 4
                                    jb = bb * KB_PER_B + k0
                                    nc.tensor.matmul(
                                        ps_o[:],
                                        v_sb[:, jb * WKK + hl * DV: jb * WKK + (hl + 1) * DV],
                                        exs[k0][:],
                                        start=(k0 == 0), stop=(k0 == nkc - 1),
                                        skip_group_check=True)
                                    if k0 % 4 == 3:
                                        # bf16 tree-sum of this 4-chunk group into
                                        # qsum (vector); replaces 4 PE matmuls
                                        g = k0 // 4
                                        t01 = att_s.tile([P, 512], BF16, tag="t01")
                                        nc.vector.tensor_add(t01[:], exs[k0 - 3][:],
                                                             exs[k0 - 2][:])
                                        t23 = att_s.tile([P, 512], BF16, tag="t23")
                                        nc.vector.tensor_add(t23[:], exs[k0 - 1][:],
                                                             exs[k0][:])
                                        if g == 0:
                                            nc.vector.tensor_add(qsum[:], t01[:], t23[:])
                                        else:
                                            grp = att_s.tile([P, 512], BF16, tag="grp")
                                            nc.vector.tensor_add(grp[:], t01[:], t23[:])
                                            nc.vector.tensor_add(qsum[:], qsum[:], grp[:])
                            after = None
                            if u == 1 and qt == QT_PER_B - 1:
                                # even heads complete -> ship overlapping odd attn
                                def after():
                                    nc.gpsimd.collective_compute(
                                        "AllToAll", mybir.AluOpType.bypass,
                                        replica_groups=RG,
                                        ins=[oa_in.opt()], outs=[oa_out.opt()])
                            fin_den, fin_a, fin_b = make_finishers(
                                hl, bb, qt, ps_o, ps_d, qsum, after)
                        if u == 2:
                            # receive the even-head AllToAll results while u3
                            # computes, so o_proj pass 1 starts without a stall
                            for g in range(2):
                                half = NCORES // 2 * TPC
                                hrow = NCORES // 2 * P
                                nc.sync.dma_start(
                                    oe_sb[:, g * half:(g + 1) * half]
                                    .rearrange("p (i c) -> p i c", i=NCORES // 2),
                                    oa_out[g * hrow:(g + 1) * hrow, :]
                                    .rearrange("(i p) c -> p i c", p=P))

                    # ===== Phase 3 (interleaved with the last qtile finalize +
                    # ===== ob AllToAll; remaining pass-1 blocks cover the
                    # ===== flight; odd-half w_o loads during it too) =====
                    att_in_stack.close()  # frees the attention k/v/q tiles
                    with tc.tile_pool(name="p3w", bufs=1) as p3w, \
                         tc.tile_pool(name="p3t", bufs=3) as p3t:
                        part_sb = p3w.tile([P, NHID * TPC], F32)
                        oo_sb = p3w.tile([P, NCORES * TPC], BF16)

                        def pass1_block(m):
                            ps = ps_s_pool.tile([P, TPC], F32, tag="pss")
                            for i in range(NCORES):
                                nc.tensor.matmul(
                                    ps[:], wo_sb[:, i * HID + m * P: i * HID + (m + 1) * P],
                                    oe_sb[:, i * TPC:(i + 1) * TPC],
                                    start=(i == 0), stop=(i == NCORES - 1))
                            nc.scalar.copy(part_sb[:, m * TPC:(m + 1) * TPC], ps[:])

                        # run ALL finishers first so the ob AllToAll fires as
                        # early as possible; pass1 then covers its flight
                        fin_den()
                        fin_den = None
                        fin_a()
                        fin_a = None
                        fin_b()
                        fin_b = None
                        nc.gpsimd.collective_compute(
                            "AllToAll", mybir.AluOpType.bypass, replica_groups=RG,
                            ins=[ob_in.opt()], outs=[ob_out.opt()])
                        # receive in quarters on two queues so pass 2 starts
                        # as soon as possible after the A2A lands
                        for g in range(4):
                            qtr = NCORES // 4 * TPC
                            qrow = NCORES // 4 * P
                            eng = nc.scalar if g % 2 == 0 else nc.sync
                            eng.dma_start(
                                oo_sb[:, g * qtr:(g + 1) * qtr]
                                .rearrange("p (i c) -> p i c", i=NCORES // 4),
                                ob_out[g * qrow:(g + 1) * qrow, :]
                                .rearrange("(i p) c -> p i c", p=P))
                        for m in range(NHID):
                            pass1_block(m)
                        for m in range(NHID):
                            ps = ps_o_pool.tile([P, TPC], F32, tag="pso")
                            for i in range(NCORES):
                                nc.tensor.matmul(
                                    ps[:], wo_sb[:, (NCORES + i) * HID + m * P:
                                                 (NCORES + i) * HID + (m + 1) * P],
                                    oo_sb[:, i * TPC:(i + 1) * TPC],
                                    start=(i == 0), stop=(i == NCORES - 1))
                            ot = p3t.tile([P, TPC], F32, tag="ot")
                            nc.vector.tensor_add(ot[:], ps[:],
                                                 part_sb[:, m * TPC:(m + 1) * TPC])
                            oeng = nc.sync if m % 2 == 0 else nc.scalar
                            oeng.dma_start(outT.ap()[m * P:(m + 1) * P, :], ot[:])
    nc.finalize()
    return nc


def _bf16(x):
    return np.ascontiguousarray(x.astype(ml_dtypes.bfloat16))


def _rope_tables():
    inv_freq = 1.0 / (THETA ** (np.arange(0, DR, 2, dtype=np.float64) / DR))
    t = np.arange(S, dtype=np.float64)
    freqs = np.outer(t, inv_freq)
    emb = np.concatenate((freqs, freqs), axis=-1)
    return np.cos(emb).astype(np.float32), np.sin(emb).astype(np.float32)


def prepare_inputs(hidden_states, w_qa, q_a_ln_w, w_qb, w_kva, kv_a_ln_w, w_kvb, w_o):
    hidden_states = np.asarray(hidden_states, dtype=np.float32)
    w_qa = np.asarray(w_qa, dtype=np.float32)
    q_a_ln_w = np.asarray(q_a_ln_w, dtype=np.float32)
    w_qb = np.asarray(w_qb, dtype=np.float32)
    w_kva = np.asarray(w_kva, dtype=np.float32)
    kv_a_ln_w = np.asarray(kv_a_ln_w, dtype=np.float32)
    w_kvb = np.asarray(w_kvb, dtype=np.float32)
    w_o = np.asarray(w_o, dtype=np.float32)

    flat = hidden_states.reshape(T, HID)
    cos, sin = _rope_tables()          # [S, DR]
    scale = DQ ** -0.5

    pos = np.arange(T) % S
    cos_d = cos[pos].T                 # [DR, T]
    sin_d = sin[pos].T

    kp = np.arange(P)[:, None]
    qf = np.arange(512)[None, :]
    # multiplicative 0/1 keep-masks (applied to exp on gpsimd, not the PE)
    masks = _bf16(np.concatenate(
        [(qf >= kp + P * p).astype(np.float32) for p in range(4)], axis=1))

    w_qb_eff = (w_qb * q_a_ln_w[None, :]) * scale       # [H*DQ, QLR]
    w_kvb_eff = w_kvb * kv_a_ln_w[None, :]              # [H*(DN+DV), KVLR]

    # w_qb rows permuted to match phase-1c block order:
    # group 0: per pair j [h0 pe | h1 pe] (pe2), group 1: h0 nope, group 2: h1 nope
    rows = []
    for j in range(NCORES):
        h0, h1 = 2 * j, 2 * j + 1
        rows.append(w_qb_eff[h0 * DQ + DN: h0 * DQ + DQ])   # h0 pe (64)
        rows.append(w_qb_eff[h1 * DQ + DN: h1 * DQ + DQ])   # h1 pe (64)
    for j in range(NCORES):
        h0 = 2 * j
        rows.append(w_qb_eff[h0 * DQ: h0 * DQ + DN])        # h0 nope (128)
    for j in range(NCORES):
        h1 = 2 * j + 1
        rows.append(w_qb_eff[h1 * DQ: h1 * DQ + DN])        # h1 nope (128)
    wqbT_full = _bf16(np.concatenate(rows, axis=0).T)       # [QLR, 3072]

    wqaT = _bf16(w_qa.T)
    wkvaT = _bf16(w_kva.T)
    woT = _bf16(w_o.T)

    in_maps = []
    for c in range(NCORES):
        heads = [HPC * c + h for h in range(HPC)]
        krows = [w_kvb_eff[h * (DN + DV): h * (DN + DV) + DN] for h in heads]
        wkvbkT_c = _bf16(np.concatenate(krows, axis=0).T)
        vrows = [w_kvb_eff[h * (DN + DV) + DN: (h + 1) * (DN + DV)] for h in heads]
        wkvbvT_c = _bf16(np.concatenate(vrows, axis=0).T)

        tok0 = c * TPC
        cosl = cos_d[:, tok0:tok0 + TPC]
        sinl = sin_d[:, tok0:tok0 + TPC]
        in_maps.append({
            "hidT": _bf16(flat[tok0:tok0 + TPC].T),
            "wqaT": wqaT, "wkvaT": wkvaT,
            "wqbT": wqbT_full, "wkvbkT": wkvbkT_c, "wkvbvT": wkvbvT_c,
            "woT": woT,
            "cosd": np.ascontiguousarray(np.concatenate([cosl, cosl], axis=0)),
            "sind": np.ascontiguousarray(np.concatenate([sinl, sinl], axis=0)),
            "masks": masks,
        })
    return in_maps


def kernel(hidden_states, w_qa, q_a_ln_w, w_qb, w_kva, kv_a_ln_w, w_kvb, w_o,
           _trace=False):
    global _NC_CACHE
    if _NC_CACHE is None:
        _NC_CACHE = build_nc()
    nc = _NC_CACHE
    in_maps = prepare_inputs(hidden_states, w_qa, q_a_ln_w, w_qb, w_kva,
                             kv_a_ln_w, w_kvb, w_o)
    res = run_bass_kernel_spmd(nc, in_maps, core_ids=list(range(NCORES)),
                               trace=_trace)
    out = np.empty((T, HID), dtype=np.float32)
    for c in range(NCORES):
        out[c * TPC:(c + 1) * TPC] = res.results[c]["outT"].T
    if _trace:
        kernel._last_result = res
    return out.reshape(B, S, HID)

